# revision 33
# baseline (speedup 1.0000x reference)
"""CGCNN (3x CGConv + global mean pool + MLP) on 8 Trainium2 NeuronCores.

Strategy (edge/graph partition parallelism):
  - Nodes are sharded into 8 contiguous ranges; each core owns all in-edges
    of its node range, so segment sums are core-local (no all-reduce needed
    for the conv; only the tiny pooled vector is all-reduced at the end).
  - Within a core, nodes are permuted by in-degree and packed into
    (group=8 blocks) x (block=128 nodes) x (D_g slots) -- a dense padded
    layout, so the per-edge message math, the segment reduction, and the
    degree normalization are all dense strided engine ops.
  - The only irregular step -- gathering per-source node table rows into
    edge slots -- is a static integer permutation of the edge list.  It is
    applied on the host between device launches (pure indexing; all
    floating-point math of the network runs on-device).
  - Per layer the device computes: z_f/z_s = A[dst] + T[src] + e*w_e,
    msg = sigmoid(z_f)*softplus(z_s), per-node slot reduction, x' = x +
    agg/deg, and the next layer's node tables T'/A' = x'@W.  The last
    launch also does the one-hot pooling matmuls, an 8-core AllReduce of
    the [5,512] pooled partials, and the 5->64->1 MLP.
"""
import sys

sys.path.insert(0, "/opt/trn_rl_repo")

import numpy as np
import ml_dtypes

from concourse import bass, mybir, tile
from concourse.bass_utils import run_bass_kernel_spmd

# ---------------------------------------------------------------------------
# Workarounds for the installed walrus's per-instruction sync-wait cap: any
# instruction carrying >1 sem waits fails codegen ("Too many sync wait
# commands").  (1) split excess waits into standalone EventSemaphore
# instructions; (2) replace the TileContext-exit drain (one wait per live
# tile semaphore) with barrier -> full-range DMA drain -> barrier.
# ---------------------------------------------------------------------------
_MAX_WAITS = 1
_wsplit_counter = [0]


def _split_excess_waits(ordered):
    for insts in ordered.values():
        out = []
        for inst in insts:
            si = inst.sync_info
            waits = list(si.on_wait) if si is not None and si.on_wait else []
            if len(waits) > _MAX_WAITS:
                keep = waits[-_MAX_WAITS:]
                for w in waits[:-_MAX_WAITS]:
                    _wsplit_counter[0] += 1
                    wi = mybir.InstEventSemaphore(
                        name=f"WSPLIT-{_wsplit_counter[0]}", ins=[], outs=[]
                    )
                    wi.engine = inst.engine
                    wi.sync_info = mybir.SyncInfo(on_wait=[w], on_update=[])
                    if inst.debug is not None:
                        wi.debug = inst.debug
                    out.append(wi)
                inst.sync_info = mybir.SyncInfo(
                    on_wait=keep,
                    on_update=list(si.on_update) if si.on_update else [],
                )
            out.append(inst)
        insts[:] = out


_orig_lower = tile.TileContext._lower_ordered_insts


def _patched_lower(self, ordered):
    _split_excess_waits(ordered)
    return _orig_lower(self, ordered)


def _patched_drain_and_barrier(self, tick_clock, wait_clock):
    nc = self.nc
    nc.all_engine_barrier()
    nc.sync.drain(semaphore_range=nc._kernel_sem_range)
    nc.all_engine_barrier()
    popped = nc._tile_sem_poison_stack.pop()
    assert popped is self._sem_poison
    nc.clear_and_free_semaphores(list(self.sems.allocated().values()))
    nc.all_engine_barrier()


if getattr(tile.TileContext, "_cgcnn_patched", False) is False:
    tile.TileContext._lower_ordered_insts = _patched_lower
    tile.TileContext._drain_and_barrier = _patched_drain_and_barrier
    tile.TileContext._cgcnn_patched = True

BF16 = mybir.dt.bfloat16
F32 = mybir.dt.float32
NPBF16 = ml_dtypes.bfloat16

N_NODES = 500_000
N_EDGES = 16_000_000
IN_DIM = 5
HIDDEN = 64
N_GRAPHS = 512
NCORES = 8
NPC = N_NODES // NCORES      # 62500 nodes per core
P = 128                      # partitions / block size
BPG = 8                      # blocks per group
NBLK = 496                   # padded blocks per core (496*128 = 63488 >= 62500)
NGRP = NBLK // BPG           # 62 groups
NPAD = NBLK * P              # padded nodes per core
PAD_VAL = -10000.0           # gathered value for padding slots -> msg == 0
WORK_BUFS = 2                # pipeline depth of per-group working tiles
ABLATE = set()               # timing-experiment flags (never set in production)

_cache = {}


def _build_wpacks(params):
    """Per-layer weight packs, replicated over 128 partitions."""
    packs = []
    for l in (1, 2, 3):
        Wf, bf = params[f"Wf{l}"], params[f"bf{l}"]
        Ws, bs = params[f"Ws{l}"], params[f"bs{l}"]
        # T coeffs: T[n,c] = sum_k x[n,k]*tc[k][c],  c in [0,10)
        tc = np.zeros((IN_DIM, 2 * IN_DIM), np.float32)
        ac = np.zeros((IN_DIM, 2 * IN_DIM), np.float32)
        for k in range(IN_DIM):
            tc[k, :IN_DIM] = Wf[:, IN_DIM + k]
            tc[k, IN_DIM:] = Ws[:, IN_DIM + k]
            ac[k, :IN_DIM] = Wf[:, k]
            ac[k, IN_DIM:] = Ws[:, k]
        abias = np.concatenate([bf, bs]).astype(np.float32)
        we = np.concatenate([Wf[:, 2 * IN_DIM], Ws[:, 2 * IN_DIM]]).astype(np.float32)
        packs.append(
            dict(
                tc=np.ascontiguousarray(tc),
                ac=np.ascontiguousarray(ac),
                abias=abias,
                we=we,
            )
        )
    return packs


def _table_build(nc, sbuf, x_t, tc_t, ac_t, ab_t, t_out, a_out):
    """T/A node tables from x (all [128, NBLK, *] resident tiles), on DVE.

    x_t: [128, NBLK, 5] f32;  tc_t/ac_t: [128, 5, 10] f32; ab_t: [128, 10]
    t_out/a_out: [128, NBLK, 10] f32
    """
    tmp = sbuf.tile([P, NBLK, 10], BF16, tag="tbl_tmp")
    for dst_t, coef, bias in ((t_out, tc_t, None), (a_out, ac_t, ab_t)):
        for k in range(IN_DIM):
            xk = x_t[:, :, k : k + 1].to_broadcast([P, NBLK, 10])
            ck = coef[:, k : k + 1, :].to_broadcast([P, NBLK, 10])
            if k == 0:
                nc.vector.tensor_tensor(
                    out=dst_t[:], in0=xk, in1=ck, op=mybir.AluOpType.mult
                )
            else:
                nc.vector.tensor_tensor(
                    out=tmp[:], in0=xk, in1=ck, op=mybir.AluOpType.mult
                )
                nc.vector.tensor_tensor(
                    out=dst_t[:], in0=dst_t[:], in1=tmp[:], op=mybir.AluOpType.add
                )
        if bias is not None:
            nc.vector.tensor_tensor(
                out=dst_t[:],
                in0=dst_t[:],
                in1=bias[:, None, :].to_broadcast([P, NBLK, 10]),
                op=mybir.AluOpType.add,
            )


def _supergroups(dgs, budget=90, maxlen=6):
    """Chunk group indices so each chunk's z tiles fit in SBUF."""
    sgs, cur, tot = [], [], 0
    for g, dg in enumerate(dgs):
        if cur and (tot + dg > budget or len(cur) >= maxlen):
            sgs.append(cur)
            cur, tot = [], 0
        cur.append(g)
        tot += dg
    if cur:
        sgs.append(cur)
    return sgs


def _build_program(mode, dgs):
    """mode: 'A' (initial tables), 'B' (conv + next tables), 'B3' (conv+pool+mlp).

    dgs: list of per-group slot counts D_g (compile-time constants).
    """
    nc = bass.Bass("TRN2", target_bir_lowering=False, debug=False, num_devices=NCORES)
    SLOTS = sum(P * BPG * d for d in dgs)  # total edge slots per core

    ins = {}

    def dram_in(name, shape, dt):
        ins[name] = nc.dram_tensor(name, shape, dt, kind="ExternalInput").ap()
        return ins[name]

    outs = {}

    def dram_out(name, shape, dt):
        outs[name] = nc.dram_tensor(name, shape, dt, kind="ExternalOutput").ap()
        return outs[name]

    x_d = dram_in("x", [P, NBLK * IN_DIM], F32)
    if mode in ("B", "B3"):
        g_d = dram_in("gat", [SLOTS * 10], BF16)   # gathered T[src] per slot
        at_d = dram_in("at10", [SLOTS * 10], BF16)  # edge_attr replicated x10
        a_d = dram_in("adst", [P, NBLK * 10], BF16)
        iv_d = dram_in("invdeg", [P, NBLK], F32)
        we_d = dram_in("wecur", [P, 10], BF16)
    if mode in ("A", "B"):
        tc_d = dram_in("tcoef", [P, IN_DIM * 10], F32)
        ac_d = dram_in("acoef", [P, IN_DIM * 10], F32)
        ab_d = dram_in("abias", [P, 10], F32)
        t_out_d = dram_out("t_next", [P, NBLK * 10], BF16)
        a_out_d = dram_out("a_next", [P, NBLK * 10], BF16)
    if mode == "B":
        x_out_d = dram_out("x_next", [P, NBLK * IN_DIM], F32)
    if mode == "B3":
        gid_d = dram_in("gid2", [P, NBLK * 2], BF16)
        iota_d = dram_in("iota", [P, N_GRAPHS // 2], BF16)
        icnt_d = dram_in("invcnt", [P, N_GRAPHS], F32)
        w1t_d = dram_in("w1t", [IN_DIM, HIDDEN], F32)
        b1_d = dram_in("b1", [HIDDEN, 1], F32)
        w2t_d = dram_in("w2t", [HIDDEN, 1], F32)
        b2_d = dram_in("b2", [1, 1], F32)
        out_d = dram_out("out", [1, N_GRAPHS], F32)

    with tile.TileContext(nc) as tc:
        with (
            tc.tile_pool(name="sbuf", bufs=1) as res,
            tc.tile_pool(name="work", bufs=WORK_BUFS) as work,
            tc.tile_pool(name="psum", bufs=2, space="PSUM") as psum,
        ):
            x_t = res.tile([P, NBLK, IN_DIM], F32)
            nc.sync.dma_start(out=x_t[:], in_=x_d[:, :])

            if mode in ("A", "B"):
                tc_t = res.tile([P, IN_DIM, 10], F32)
                ac_t = res.tile([P, IN_DIM, 10], F32)
                ab_t = res.tile([P, 10], F32)
                nc.sync.dma_start(out=tc_t[:], in_=tc_d[:, :])
                nc.sync.dma_start(out=ac_t[:], in_=ac_d[:, :])
                nc.sync.dma_start(out=ab_t[:], in_=ab_d[:, :])

            if mode == "A":
                t_o = res.tile([P, NBLK, 10], BF16)
                a_o = res.tile([P, NBLK, 10], BF16)
                _table_build(nc, res, x_t, tc_t, ac_t, ab_t, t_o, a_o)
                nc.sync.dma_start(out=t_out_d[:, :], in_=t_o[:])
                nc.sync.dma_start(out=a_out_d[:, :], in_=a_o[:])
            else:
                a_t = res.tile([P, NBLK, 10], BF16)
                iv_t = res.tile([P, NBLK], F32)
                we_t = res.tile([P, 10], BF16)
                nc.sync.dma_start(out=a_t[:], in_=a_d[:, :])
                nc.sync.dma_start(out=iv_t[:], in_=iv_d[:, :])
                nc.sync.dma_start(out=we_t[:], in_=we_d[:, :])
                xn_t = res.tile([P, NBLK, IN_DIM], F32)

                if mode == "B3":
                    gid_t = res.tile([P, NBLK, 2], BF16)
                    iota_t = res.tile([P, N_GRAPHS // 2], BF16)
                    nc.sync.dma_start(out=gid_t[:], in_=gid_d[:, :])
                    nc.sync.dma_start(out=iota_t[:], in_=iota_d[:, :])
                    pool_ps = psum.tile([IN_DIM, N_GRAPHS], F32)

                offs, off = [], 0
                for dg in dgs:
                    offs.append(off)
                    off += P * BPG * dg
                sgs = _supergroups(dgs)

                with tc.tile_pool(name="zpool", bufs=1) as zpool:
                    for sg_groups in sgs:
                        ztiles = {}
                        # phase 1: z = at10*we + gat + A[dst]  (DVE, all 2x)
                        for zi, g in enumerate(sg_groups):
                            dg = dgs[g]
                            gape = [
                                (BPG * dg * 10, P), (dg * 10, BPG), (10, dg), (1, 10),
                            ]
                            g_ap = bass.AP(g_d.tensor, offs[g] * 10, gape)
                            at_ap = bass.AP(at_d.tensor, offs[g] * 10, gape)
                            z = zpool.tile([P, BPG, dg, 10], BF16, tag=f"z{zi}")
                            gt = work.tile([P, BPG, dg, 10], BF16, tag="gat")
                            nc.sync.dma_start(out=z[:], in_=at_ap)
                            nc.sync.dma_start(out=gt[:], in_=g_ap)
                            nc.vector.tensor_tensor(
                                out=z[:],
                                in0=z[:],
                                in1=we_t[:, None, None, :].to_broadcast(
                                    [P, BPG, dg, 10]
                                ),
                                op=mybir.AluOpType.mult,
                            )
                            nc.vector.tensor_tensor(
                                out=z[:], in0=z[:], in1=gt[:], op=mybir.AluOpType.add
                            )
                            nc.vector.tensor_tensor(
                                out=z[:],
                                in0=z[:],
                                in1=a_t[
                                    :, g * BPG : (g + 1) * BPG, None, :
                                ].to_broadcast([P, BPG, dg, 10]),
                                op=mybir.AluOpType.add,
                            )
                            ztiles[g] = z
                        # phase 2: zf <- sigmoid(zf)   (ACT, sigmoid table)
                        for g in sg_groups:
                            z = ztiles[g]
                            nc.scalar.activation(
                                out=z[:, :, :, 0:IN_DIM],
                                in_=z[:, :, :, 0:IN_DIM],
                                func=mybir.ActivationFunctionType.Sigmoid,
                            )
                        # phase 3: zs <- ln(exp(zs)+1)  (ACT, nl_exp table)
                        for g in sg_groups:
                            z = ztiles[g]
                            nc.scalar.activation(
                                out=z[:, :, :, IN_DIM : 2 * IN_DIM],
                                in_=z[:, :, :, IN_DIM : 2 * IN_DIM],
                                func=mybir.ActivationFunctionType.Exp,
                            )
                            nc.scalar.activation(
                                out=z[:, :, :, IN_DIM : 2 * IN_DIM],
                                in_=z[:, :, :, IN_DIM : 2 * IN_DIM],
                                func=mybir.ActivationFunctionType.Ln,
                                bias=1.0,
                            )
                        # phase 4: m = sg*sp (Pool), segment reduce (DVE), x'
                        for g in sg_groups:
                            z = ztiles[g]
                            dg = dgs[g]
                            nc.gpsimd.tensor_tensor(
                                out=z[:, :, :, IN_DIM : 2 * IN_DIM],
                                in0=z[:, :, :, 0:IN_DIM],
                                in1=z[:, :, :, IN_DIM : 2 * IN_DIM],
                                op=mybir.AluOpType.mult,
                            )
                            zv = z[:]
                            m_view = bass.AP(
                                zv.tensor,
                                zv.offset + IN_DIM,
                                [zv.ap[0], (dg * 10, BPG), (1, IN_DIM), (10, dg)],
                            )
                            agg = work.tile([P, BPG, IN_DIM], F32, tag="agg")
                            nc.vector.tensor_reduce(
                                out=agg[:],
                                in_=m_view,
                                axis=mybir.AxisListType.X,
                                op=mybir.AluOpType.add,
                            )
                            sc = work.tile([P, BPG, IN_DIM], F32, tag="sc")
                            nc.vector.tensor_tensor(
                                out=sc[:],
                                in0=agg[:],
                                in1=iv_t[
                                    :, g * BPG : (g + 1) * BPG, None
                                ].to_broadcast([P, BPG, IN_DIM]),
                                op=mybir.AluOpType.mult,
                            )
                            nc.vector.tensor_tensor(
                                out=xn_t[:, g * BPG : (g + 1) * BPG, :],
                                in0=sc[:],
                                in1=x_t[:, g * BPG : (g + 1) * BPG, :],
                                op=mybir.AluOpType.add,
                            )

                            if mode == "B3":
                                xnb = work.tile([P, BPG, IN_DIM], BF16, tag="xnb")
                                nc.vector.tensor_copy(
                                    out=xnb[:], in_=xn_t[:, g * BPG : (g + 1) * BPG, :]
                                )
                                half = N_GRAPHS // 2
                                ohg = work.tile([P, BPG, N_GRAPHS], BF16, tag="ohg")
                                for h in (0, 1):
                                    nc.vector.tensor_tensor(
                                        out=ohg[:, :, h * half : (h + 1) * half],
                                        in0=gid_t[
                                            :, g * BPG : (g + 1) * BPG, h : h + 1
                                        ].to_broadcast([P, BPG, half]),
                                        in1=iota_t[:, None, :].to_broadcast(
                                            [P, BPG, half]
                                        ),
                                        op=mybir.AluOpType.is_equal,
                                    )
                                for bb in range(BPG):
                                    blk = g * BPG + bb
                                    nc.tensor.matmul(
                                        out=pool_ps[:],
                                        lhsT=xnb[:, bb, :],
                                        rhs=ohg[:, bb, :],
                                        start=(blk == 0),
                                        stop=(blk == NBLK - 1),
                                    )

                if mode == "B":
                    nc.sync.dma_start(out=x_out_d[:, :], in_=xn_t[:])
                    t_o = res.tile([P, NBLK, 10], BF16)
                    a_o = res.tile([P, NBLK, 10], BF16)
                    _table_build(nc, res, xn_t, tc_t, ac_t, ab_t, t_o, a_o)
                    nc.sync.dma_start(out=t_out_d[:, :], in_=t_o[:])
                    nc.sync.dma_start(out=a_out_d[:, :], in_=a_o[:])
                else:
                    # pooled partial sums -> scale -> AllReduce -> MLP
                    icnt_t = res.tile([P, N_GRAPHS], F32)
                    nc.sync.dma_start(out=icnt_t[:], in_=icnt_d[:, :])
                    psum_sb = res.tile([IN_DIM, N_GRAPHS], F32)
                    nc.vector.tensor_tensor(
                        out=psum_sb[:],
                        in0=pool_ps[:],
                        in1=icnt_t[0:IN_DIM, :],
                        op=mybir.AluOpType.mult,
                    )
                    with tc.tile_pool(name="dram", bufs=1, space="DRAM") as dpool:
                        cc_in = dpool.tile([IN_DIM, N_GRAPHS], F32)
                        cc_out = dpool.tile([IN_DIM, N_GRAPHS], F32)
                        nc.sync.dma_start(out=cc_in[:], in_=psum_sb[:])
                        nc.gpsimd.collective_compute(
                            "AllReduce",
                            mybir.AluOpType.add,
                            replica_groups=[list(range(NCORES))],
                            ins=[cc_in.opt()],
                            outs=[cc_out.opt()],
                        )
                        pooled_t = res.tile([IN_DIM, N_GRAPHS], F32)
                        nc.sync.dma_start(out=pooled_t[:], in_=cc_out[:])
                    w1t_t = res.tile([IN_DIM, HIDDEN], F32)
                    b1_t = res.tile([HIDDEN, 1], F32)
                    w2t_t = res.tile([HIDDEN, 1], F32)
                    b2_t = res.tile([1, 1], F32)
                    nc.sync.dma_start(out=w1t_t[:], in_=w1t_d[:, :])
                    nc.sync.dma_start(out=b1_t[:], in_=b1_d[:, :])
                    nc.sync.dma_start(out=w2t_t[:], in_=w2t_d[:, :])
                    nc.sync.dma_start(out=b2_t[:], in_=b2_d[:, :])
                    h_ps = psum.tile([HIDDEN, N_GRAPHS], F32)
                    nc.tensor.matmul(
                        out=h_ps[:], lhsT=w1t_t[:], rhs=pooled_t[:],
                        start=True, stop=True,
                    )
                    h_t = res.tile([HIDDEN, N_GRAPHS], F32)
                    nc.scalar.activation(
                        out=h_t[:], in_=h_ps[:],
                        func=mybir.ActivationFunctionType.Relu,
                        bias=b1_t[:],
                    )
                    o_ps = psum.tile([1, N_GRAPHS], F32)
                    nc.tensor.matmul(
                        out=o_ps[:], lhsT=w2t_t[:], rhs=h_t[:],
                        start=True, stop=True,
                    )
                    o_t = res.tile([1, N_GRAPHS], F32)
                    nc.vector.tensor_tensor(
                        out=o_t[:],
                        in0=o_ps[:],
                        in1=b2_t[:, 0:1].to_broadcast([1, N_GRAPHS]),
                        op=mybir.AluOpType.add,
                    )
                    nc.sync.dma_start(out=out_d[:, :], in_=o_t[:])
    return nc


def _preprocess(edge_index, edge_attr, batch):
    src = np.asarray(edge_index[0], dtype=np.int64)
    dst = np.asarray(edge_index[1], dtype=np.int64)
    attr = np.asarray(edge_attr, dtype=np.float32).reshape(-1)
    batch = np.asarray(batch, dtype=np.int64)

    deg = np.bincount(dst, minlength=N_NODES)
    eorder = np.argsort(dst, kind="stable")
    ssrc = src[eorder].astype(np.int32)
    sattr = attr[eorder]
    starts = np.zeros(N_NODES + 1, np.int64)
    np.cumsum(deg, out=starts[1:])

    cores = []
    for c in range(NCORES):
        lo = c * NPC
        degs = deg[lo : lo + NPC]
        order = np.argsort(-degs, kind="stable")  # local ids by degree desc
        node_arr = np.full(NPAD, -1, np.int64)
        node_arr[:NPC] = lo + order
        node_arr = node_arr.reshape(NBLK, P)  # [block, p]
        valid_n = node_arr >= 0
        nd = np.where(valid_n, deg[np.clip(node_arr, 0, None)], 0)

        dgs, src_chunks, attr_chunks = [], [], []
        for g in range(NGRP):
            nb = node_arr[g * BPG : (g + 1) * BPG]          # [BPG, P]
            db = nd[g * BPG : (g + 1) * BPG]                # [BPG, P]
            dg = max(1, int(db.max()))
            dgs.append(dg)
            j = np.arange(dg)
            eid = starts[np.clip(nb, 0, None)][:, :, None] + j  # [BPG,P,dg]
            ok = j[None, None, :] < db[:, :, None]
            sg = np.where(ok, ssrc[np.clip(eid, 0, N_EDGES - 1)], N_NODES)
            ag = np.where(ok, sattr[np.clip(eid, 0, N_EDGES - 1)], 0.0)
            src_chunks.append(sg.transpose(1, 0, 2).ravel())   # [P,BPG,dg]
            attr_chunks.append(ag.transpose(1, 0, 2).ravel().astype(NPBF16))

        slotsrc = np.concatenate(src_chunks)
        slotattr = np.concatenate(attr_chunks)
        invdeg = (1.0 / np.maximum(nd, 1)).astype(np.float32).reshape(NBLK, P).T
        invdeg = np.ascontiguousarray(invdeg)                  # [P, NBLK]
        gid = np.where(valid_n, batch[np.clip(node_arr, 0, None)], 0)
        gid = np.ascontiguousarray(gid.reshape(NBLK, P).T.astype(np.float32))
        cores.append(
            dict(
                node_arr=node_arr, dgs=dgs, slotsrc=slotsrc,
                slotattr=slotattr, invdeg=invdeg, gid=gid,
            )
        )
    cnt = np.bincount(batch, minlength=N_GRAPHS).astype(np.float32)
    invcnt = (1.0 / np.maximum(cnt, 1.0)).astype(np.float32)
    return cores, invcnt


def _node_layout(arr_full, node_arr, width):
    """[N_NODES(+), width] full-array -> per-core [P, NBLK*width] layout."""
    out = np.zeros((NBLK, P, width), np.float32)
    v = node_arr >= 0
    out[v] = arr_full[node_arr[v]]
    return np.ascontiguousarray(out.transpose(1, 0, 2).reshape(P, NBLK * width))


def _unlayout(per_core, node_arr, width):
    """[P, NBLK*width] device layout -> rows in original node ids (full)."""
    a = per_core.reshape(P, NBLK, width).transpose(1, 0, 2)  # [NBLK, P, w]
    return a, node_arr


def kernel(x, edge_index, edge_attr, batch, **params):
    x = np.asarray(x, dtype=np.float32)
    cores, invcnt = _preprocess(edge_index, edge_attr, batch)
    wpacks = _build_wpacks(params)

    dgs = cores[0]["dgs"]
    # all cores must share one compiled program -> unify group sizes
    max_dgs = [max(c["dgs"][g] for c in cores) for g in range(NGRP)]
    for c in cores:
        if c["dgs"] != max_dgs:
            # repack with padded group sizes
            c_new_src, c_new_attr, off = [], [], 0
            for g, (dg_c, dg_m) in enumerate(zip(c["dgs"], max_dgs)):
                n = P * BPG * dg_c
                sg = c["slotsrc"][off : off + n].reshape(P, BPG, dg_c)
                ag = c["slotattr"][off : off + n].reshape(P, BPG, dg_c)
                off += n
                if dg_c < dg_m:
                    sg2 = np.full((P, BPG, dg_m), N_NODES, np.int32)
                    ag2 = np.zeros((P, BPG, dg_m), NPBF16)
                    sg2[:, :, :dg_c] = sg
                    ag2[:, :, :dg_c] = ag
                    sg, ag = sg2, ag2
                c_new_src.append(sg.ravel())
                c_new_attr.append(ag.ravel())
            c["slotsrc"] = np.concatenate(c_new_src)
            c["slotattr"] = np.concatenate(c_new_attr)
            c["dgs"] = list(max_dgs)
    dgs = max_dgs
    S = sum(P * BPG * d for d in dgs)
    for c in cores:
        c["at10"] = np.repeat(c["slotattr"], 10)

    key = tuple(dgs)
    if key not in _cache:
        _cache[key] = (
            _build_program("A", dgs),
            _build_program("B", dgs),
            _build_program("B3", dgs),
        )
    nc_a, nc_b, nc_b3 = _cache[key]

    import os

    core_ids = list(range(NCORES))
    if bool(int(os.environ.get("CGCNN_TRACE", "0"))):
        from concourse.timeline_sim import TimelineSim

        est = []
        for name, prog in (("A", nc_a), ("B", nc_b), ("B3", nc_b3)):
            try:
                ns = TimelineSim(prog).simulate()
            except Exception as e:
                ns = None
                print(f"TimelineSim {name} failed: {e}")
            est.append((name, ns))
        kernel.timeline_est = est
        print("cost-model timeline estimate per launch (ns):", est)
    rep = np.ones((P, 1), np.float32)

    def wp_maps(l):
        wp = wpacks[l]
        return {
            "tcoef": np.ascontiguousarray((rep * wp["tc"].reshape(1, -1))),
            "acoef": np.ascontiguousarray((rep * wp["ac"].reshape(1, -1))),
            "abias": np.ascontiguousarray((rep * wp["abias"].reshape(1, -1))),
        }

    # ---- launch A: initial tables from x ----
    in_maps = []
    for c in cores:
        m = {"x": _node_layout(x, c["node_arr"], IN_DIM)}
        m.update(wp_maps(0))
        in_maps.append(m)
    res_a = run_bass_kernel_spmd(nc_a, in_maps, core_ids=core_ids)

    exec_times = [res_a.exec_time_ns]
    trace_paths = []
    if res_a.instructions_and_trace is not None:
        trace_paths.append(res_a.instructions_and_trace[1])

    def assemble_tables(res):
        t_full = np.full((N_NODES + 1, 10), PAD_VAL, NPBF16)
        a_slices = []
        for ci, c in enumerate(cores):
            t_pc, node_arr = _unlayout(res.results[ci]["t_next"], c["node_arr"], 10)
            v = node_arr >= 0
            t_full[node_arr[v]] = t_pc[v]
            a_slices.append(np.ascontiguousarray(res.results[ci]["a_next"]))
        return t_full, a_slices

    t_full, a_slices = assemble_tables(res_a)

    x_cur = [in_maps[c]["x"] for c in range(NCORES)]

    for l in (1, 2, 3):
        last = l == 3
        in_maps = []
        for ci, c in enumerate(cores):
            gat = np.ascontiguousarray(t_full[c["slotsrc"]]).ravel()
            wp = wpacks[l - 1]
            m = {
                "x": x_cur[ci],
                "gat": gat,
                "at10": c["at10"],
                "adst": a_slices[ci],
                "invdeg": c["invdeg"],
                "wecur": np.ascontiguousarray(
                    (rep * wp["we"].reshape(1, -1))
                ).astype(NPBF16),
            }
            if not last:
                m.update(wp_maps(l))
            else:
                g2 = np.stack([c["gid"], c["gid"] - 256.0], axis=-1)
                m["gid2"] = np.ascontiguousarray(
                    g2.reshape(P, NBLK * 2)
                ).astype(NPBF16)
                m["iota"] = np.ascontiguousarray(
                    rep * np.arange(N_GRAPHS // 2, dtype=np.float32).reshape(1, -1)
                ).astype(NPBF16)
                m["invcnt"] = np.ascontiguousarray(
                    rep * invcnt.reshape(1, -1)
                )
                m["w1t"] = np.ascontiguousarray(params["W1"].T.astype(np.float32))
                m["b1"] = np.ascontiguousarray(
                    params["b1"].astype(np.float32).reshape(HIDDEN, 1)
                )
                m["w2t"] = np.ascontiguousarray(params["W2"].T.astype(np.float32))
                m["b2"] = np.ascontiguousarray(
                    params["b2"].astype(np.float32).reshape(1, 1)
                )
            in_maps.append(m)
        res = run_bass_kernel_spmd(
            nc_b3 if last else nc_b, in_maps, core_ids=core_ids
        )
        exec_times.append(res.exec_time_ns)
        if res.instructions_and_trace is not None:
            trace_paths.append(res.instructions_and_trace[1])
        if not last:
            t_full, a_slices = assemble_tables(res)
            x_cur = [res.results[ci]["x_next"] for ci in range(NCORES)]

    kernel.exec_times = exec_times
    kernel.trace_paths = trace_paths
    out = res.results[0]["out"].reshape(N_GRAPHS, 1).astype(np.float32)
    return out



# revision 36
# speedup vs baseline: 1.0182x; 1.0182x over previous
"""CGCNN (3x CGConv + global mean pool + MLP) on 8 Trainium2 NeuronCores.

Strategy (edge/graph partition parallelism):
  - Nodes are sharded into 8 contiguous ranges; each core owns all in-edges
    of its node range, so segment sums are core-local (no all-reduce needed
    for the conv; only the tiny pooled vector is all-reduced at the end).
  - Within a core, nodes are permuted by in-degree and packed into
    (group=8 blocks) x (block=128 nodes) x (D_g slots) -- a dense padded
    layout, so the per-edge message math, the segment reduction, and the
    degree normalization are all dense strided engine ops.
  - The only irregular step -- gathering per-source node table rows into
    edge slots -- is a static integer permutation of the edge list.  It is
    applied on the host between device launches (pure indexing; all
    floating-point math of the network runs on-device).
  - Per layer the device computes: z_f/z_s = A[dst] + T[src] + e*w_e,
    msg = sigmoid(z_f)*softplus(z_s), per-node slot reduction, x' = x +
    agg/deg, and the next layer's node tables T'/A' = x'@W.  The last
    launch also does the one-hot pooling matmuls, an 8-core AllReduce of
    the [5,512] pooled partials, and the 5->64->1 MLP.
"""
import sys

sys.path.insert(0, "/opt/trn_rl_repo")

import numpy as np
import ml_dtypes

from concourse import bass, mybir, tile
from concourse.bass_utils import run_bass_kernel_spmd

# ---------------------------------------------------------------------------
# Workarounds for the installed walrus's per-instruction sync-wait cap: any
# instruction carrying >1 sem waits fails codegen ("Too many sync wait
# commands").  (1) split excess waits into standalone EventSemaphore
# instructions; (2) replace the TileContext-exit drain (one wait per live
# tile semaphore) with barrier -> full-range DMA drain -> barrier.
# ---------------------------------------------------------------------------
_MAX_WAITS = 1
_wsplit_counter = [0]


def _split_excess_waits(ordered):
    for insts in ordered.values():
        out = []
        for inst in insts:
            si = inst.sync_info
            waits = list(si.on_wait) if si is not None and si.on_wait else []
            if len(waits) > _MAX_WAITS:
                keep = waits[-_MAX_WAITS:]
                for w in waits[:-_MAX_WAITS]:
                    _wsplit_counter[0] += 1
                    wi = mybir.InstEventSemaphore(
                        name=f"WSPLIT-{_wsplit_counter[0]}", ins=[], outs=[]
                    )
                    wi.engine = inst.engine
                    wi.sync_info = mybir.SyncInfo(on_wait=[w], on_update=[])
                    if inst.debug is not None:
                        wi.debug = inst.debug
                    out.append(wi)
                inst.sync_info = mybir.SyncInfo(
                    on_wait=keep,
                    on_update=list(si.on_update) if si.on_update else [],
                )
            out.append(inst)
        insts[:] = out


_orig_lower = tile.TileContext._lower_ordered_insts


def _patched_lower(self, ordered):
    _split_excess_waits(ordered)
    return _orig_lower(self, ordered)


def _patched_drain_and_barrier(self, tick_clock, wait_clock):
    nc = self.nc
    nc.all_engine_barrier()
    nc.sync.drain(semaphore_range=nc._kernel_sem_range)
    nc.all_engine_barrier()
    popped = nc._tile_sem_poison_stack.pop()
    assert popped is self._sem_poison
    nc.clear_and_free_semaphores(list(self.sems.allocated().values()))
    nc.all_engine_barrier()


if getattr(tile.TileContext, "_cgcnn_patched", False) is False:
    tile.TileContext._lower_ordered_insts = _patched_lower
    tile.TileContext._drain_and_barrier = _patched_drain_and_barrier
    tile.TileContext._cgcnn_patched = True

BF16 = mybir.dt.bfloat16
F32 = mybir.dt.float32
NPBF16 = ml_dtypes.bfloat16

N_NODES = 500_000
N_EDGES = 16_000_000
IN_DIM = 5
HIDDEN = 64
N_GRAPHS = 512
NCORES = 8
NPC = N_NODES // NCORES      # 62500 nodes per core
P = 128                      # partitions / block size
BPG = 8                      # blocks per group
NBLK = 496                   # padded blocks per core (496*128 = 63488 >= 62500)
NGRP = NBLK // BPG           # 62 groups
NPAD = NBLK * P              # padded nodes per core
PAD_VAL = -10000.0           # gathered value for padding slots -> msg == 0
WORK_BUFS = 3                # pipeline depth of per-group working tiles
ABLATE = set()               # timing-experiment flags (never set in production)

_cache = {}


def _build_wpacks(params):
    """Per-layer weight packs, replicated over 128 partitions."""
    packs = []
    for l in (1, 2, 3):
        Wf, bf = params[f"Wf{l}"], params[f"bf{l}"]
        Ws, bs = params[f"Ws{l}"], params[f"bs{l}"]
        # T coeffs: T[n,c] = sum_k x[n,k]*tc[k][c],  c in [0,10)
        tc = np.zeros((IN_DIM, 2 * IN_DIM), np.float32)
        ac = np.zeros((IN_DIM, 2 * IN_DIM), np.float32)
        for k in range(IN_DIM):
            tc[k, :IN_DIM] = Wf[:, IN_DIM + k]
            tc[k, IN_DIM:] = Ws[:, IN_DIM + k]
            ac[k, :IN_DIM] = Wf[:, k]
            ac[k, IN_DIM:] = Ws[:, k]
        abias = np.concatenate([bf, bs]).astype(np.float32)
        we = np.concatenate([Wf[:, 2 * IN_DIM], Ws[:, 2 * IN_DIM]]).astype(np.float32)
        packs.append(
            dict(
                tc=np.ascontiguousarray(tc),
                ac=np.ascontiguousarray(ac),
                abias=abias,
                we=we,
            )
        )
    return packs


def _table_build(nc, sbuf, x_t, tc_t, ac_t, ab_t, t_out, a_out):
    """T/A node tables from x (all [128, NBLK, *] resident tiles), on DVE.

    x_t: [128, NBLK, 5] f32;  tc_t/ac_t: [128, 5, 10] f32; ab_t: [128, 10]
    t_out/a_out: [128, NBLK, 10] f32
    """
    tmp = sbuf.tile([P, NBLK, 10], BF16, tag="tbl_tmp")
    tmp2 = sbuf.tile([P, NBLK, 10], BF16, tag="tbl_tmp2")
    for dst_t, coef, bias, eng, scr in (
        (t_out, tc_t, None, nc.vector, tmp),
        (a_out, ac_t, ab_t, nc.gpsimd, tmp2),
    ):
        for k in range(IN_DIM):
            xk = x_t[:, :, k : k + 1].to_broadcast([P, NBLK, 10])
            ck = coef[:, k : k + 1, :].to_broadcast([P, NBLK, 10])
            if k == 0:
                eng.tensor_tensor(
                    out=dst_t[:], in0=xk, in1=ck, op=mybir.AluOpType.mult
                )
            else:
                eng.tensor_tensor(
                    out=scr[:], in0=xk, in1=ck, op=mybir.AluOpType.mult
                )
                eng.tensor_tensor(
                    out=dst_t[:], in0=dst_t[:], in1=scr[:], op=mybir.AluOpType.add
                )
        if bias is not None:
            eng.tensor_tensor(
                out=dst_t[:],
                in0=dst_t[:],
                in1=bias[:, None, :].to_broadcast([P, NBLK, 10]),
                op=mybir.AluOpType.add,
            )


def _supergroups(dgs, budget=110, maxlen=7):
    """Chunk group indices so each chunk's z tiles fit in SBUF."""
    sgs, cur, tot = [], [], 0
    for g, dg in enumerate(dgs):
        if cur and (tot + dg > budget or len(cur) >= maxlen):
            sgs.append(cur)
            cur, tot = [], 0
        cur.append(g)
        tot += dg
    if cur:
        sgs.append(cur)
    return sgs


def _build_program(mode, dgs):
    """mode: 'A' (initial tables), 'B' (conv + next tables), 'B3' (conv+pool+mlp).

    dgs: list of per-group slot counts D_g (compile-time constants).
    """
    nc = bass.Bass("TRN2", target_bir_lowering=False, debug=False, num_devices=NCORES)
    SLOTS = sum(P * BPG * d for d in dgs)  # total edge slots per core

    ins = {}

    def dram_in(name, shape, dt):
        ins[name] = nc.dram_tensor(name, shape, dt, kind="ExternalInput").ap()
        return ins[name]

    outs = {}

    def dram_out(name, shape, dt):
        outs[name] = nc.dram_tensor(name, shape, dt, kind="ExternalOutput").ap()
        return outs[name]

    x_d = dram_in("x", [P, NBLK * IN_DIM], F32)
    if mode in ("B", "B3"):
        g_d = dram_in("gat", [SLOTS * 10], BF16)   # gathered T[src] per slot
        at_d = dram_in("at10", [SLOTS * 10], BF16)  # edge_attr replicated x10
        a_d = dram_in("adst", [P, NBLK * 10], BF16)
        iv_d = dram_in("invdeg", [P, NBLK], F32)
        we_d = dram_in("wecur", [P, 10], BF16)
    if mode in ("A", "B"):
        tc_d = dram_in("tcoef", [P, IN_DIM * 10], F32)
        ac_d = dram_in("acoef", [P, IN_DIM * 10], F32)
        ab_d = dram_in("abias", [P, 10], F32)
        t_out_d = dram_out("t_next", [P, NBLK * 10], BF16)
        a_out_d = dram_out("a_next", [P, NBLK * 10], BF16)
    if mode == "B":
        x_out_d = dram_out("x_next", [P, NBLK * IN_DIM], F32)
    if mode == "B3":
        gid_d = dram_in("gid2", [P, NBLK * 2], BF16)
        iota_d = dram_in("iota", [P, N_GRAPHS // 2], BF16)
        icnt_d = dram_in("invcnt", [P, N_GRAPHS], F32)
        w1t_d = dram_in("w1t", [IN_DIM, HIDDEN], F32)
        b1_d = dram_in("b1", [HIDDEN, 1], F32)
        w2t_d = dram_in("w2t", [HIDDEN, 1], F32)
        b2_d = dram_in("b2", [1, 1], F32)
        out_d = dram_out("out", [1, N_GRAPHS], F32)

    with tile.TileContext(nc) as tc:
        with (
            tc.tile_pool(name="sbuf", bufs=1) as res,
            tc.tile_pool(name="work", bufs=WORK_BUFS) as work,
            tc.tile_pool(name="psum", bufs=2, space="PSUM") as psum,
        ):
            x_t = res.tile([P, NBLK, IN_DIM], F32)
            nc.sync.dma_start(out=x_t[:], in_=x_d[:, :])

            if mode in ("A", "B"):
                tc_t = res.tile([P, IN_DIM, 10], F32)
                ac_t = res.tile([P, IN_DIM, 10], F32)
                ab_t = res.tile([P, 10], F32)
                nc.sync.dma_start(out=tc_t[:], in_=tc_d[:, :])
                nc.sync.dma_start(out=ac_t[:], in_=ac_d[:, :])
                nc.sync.dma_start(out=ab_t[:], in_=ab_d[:, :])

            if mode == "A":
                t_o = res.tile([P, NBLK, 10], BF16)
                a_o = res.tile([P, NBLK, 10], BF16)
                _table_build(nc, res, x_t, tc_t, ac_t, ab_t, t_o, a_o)
                nc.sync.dma_start(out=t_out_d[:, :], in_=t_o[:])
                nc.sync.dma_start(out=a_out_d[:, :], in_=a_o[:])
            else:
                a_t = res.tile([P, NBLK, 10], BF16)
                iv_t = res.tile([P, NBLK], F32)
                we_t = res.tile([P, 10], BF16)
                nc.sync.dma_start(out=a_t[:], in_=a_d[:, :])
                nc.sync.dma_start(out=iv_t[:], in_=iv_d[:, :])
                nc.sync.dma_start(out=we_t[:], in_=we_d[:, :])
                xn_t = res.tile([P, NBLK, IN_DIM], F32)

                if mode == "B3":
                    gid_t = res.tile([P, NBLK, 2], BF16)
                    iota_t = res.tile([P, N_GRAPHS // 2], BF16)
                    nc.sync.dma_start(out=gid_t[:], in_=gid_d[:, :])
                    nc.sync.dma_start(out=iota_t[:], in_=iota_d[:, :])
                    pool_ps = psum.tile([IN_DIM, N_GRAPHS], F32)

                offs, off = [], 0
                for dg in dgs:
                    offs.append(off)
                    off += P * BPG * dg
                sgs = _supergroups(dgs)

                with tc.tile_pool(name="zpool", bufs=1) as zpool:
                    for sg_groups in sgs:
                        ztiles = {}
                        # phase 1: z = at10*we + gat + A[dst]  (DVE, all 2x)
                        for zi, g in enumerate(sg_groups):
                            dg = dgs[g]
                            gape = [
                                (BPG * dg * 10, P), (dg * 10, BPG), (10, dg), (1, 10),
                            ]
                            g_ap = bass.AP(g_d.tensor, offs[g] * 10, gape)
                            at_ap = bass.AP(at_d.tensor, offs[g] * 10, gape)
                            z = zpool.tile([P, BPG, dg, 10], BF16, tag=f"z{zi}")
                            gt = work.tile([P, BPG, dg, 10], BF16, tag="gat")
                            nc.sync.dma_start(out=z[:], in_=at_ap)
                            nc.sync.dma_start(out=gt[:], in_=g_ap)
                            nc.vector.tensor_tensor(
                                out=z[:],
                                in0=z[:],
                                in1=we_t[:, None, None, :].to_broadcast(
                                    [P, BPG, dg, 10]
                                ),
                                op=mybir.AluOpType.mult,
                            )
                            nc.vector.tensor_tensor(
                                out=z[:], in0=z[:], in1=gt[:], op=mybir.AluOpType.add
                            )
                            nc.vector.tensor_tensor(
                                out=z[:],
                                in0=z[:],
                                in1=a_t[
                                    :, g * BPG : (g + 1) * BPG, None, :
                                ].to_broadcast([P, BPG, dg, 10]),
                                op=mybir.AluOpType.add,
                            )
                            ztiles[g] = z
                        # phase 2: zf <- sigmoid(zf)   (ACT, sigmoid table)
                        for g in sg_groups:
                            z = ztiles[g]
                            nc.scalar.activation(
                                out=z[:, :, :, 0:IN_DIM],
                                in_=z[:, :, :, 0:IN_DIM],
                                func=mybir.ActivationFunctionType.Sigmoid,
                            )
                        # phase 3: zs <- ln(exp(zs)+1)  (ACT, nl_exp table)
                        for g in sg_groups:
                            z = ztiles[g]
                            nc.scalar.activation(
                                out=z[:, :, :, IN_DIM : 2 * IN_DIM],
                                in_=z[:, :, :, IN_DIM : 2 * IN_DIM],
                                func=mybir.ActivationFunctionType.Exp,
                            )
                            nc.scalar.activation(
                                out=z[:, :, :, IN_DIM : 2 * IN_DIM],
                                in_=z[:, :, :, IN_DIM : 2 * IN_DIM],
                                func=mybir.ActivationFunctionType.Ln,
                                bias=1.0,
                            )
                        # phase 4: m = sg*sp (Pool), segment reduce (DVE), x'
                        for g in sg_groups:
                            z = ztiles[g]
                            dg = dgs[g]
                            nc.gpsimd.tensor_tensor(
                                out=z[:, :, :, IN_DIM : 2 * IN_DIM],
                                in0=z[:, :, :, 0:IN_DIM],
                                in1=z[:, :, :, IN_DIM : 2 * IN_DIM],
                                op=mybir.AluOpType.mult,
                            )
                            zv = z[:]
                            m_view = bass.AP(
                                zv.tensor,
                                zv.offset + IN_DIM,
                                [zv.ap[0], (dg * 10, BPG), (1, IN_DIM), (10, dg)],
                            )
                            agg = work.tile([P, BPG, IN_DIM], F32, tag="agg")
                            nc.vector.tensor_reduce(
                                out=agg[:],
                                in_=m_view,
                                axis=mybir.AxisListType.X,
                                op=mybir.AluOpType.add,
                            )
                            sc = work.tile([P, BPG, IN_DIM], F32, tag="sc")
                            nc.vector.tensor_tensor(
                                out=sc[:],
                                in0=agg[:],
                                in1=iv_t[
                                    :, g * BPG : (g + 1) * BPG, None
                                ].to_broadcast([P, BPG, IN_DIM]),
                                op=mybir.AluOpType.mult,
                            )
                            nc.vector.tensor_tensor(
                                out=xn_t[:, g * BPG : (g + 1) * BPG, :],
                                in0=sc[:],
                                in1=x_t[:, g * BPG : (g + 1) * BPG, :],
                                op=mybir.AluOpType.add,
                            )

                            if mode == "B3":
                                xnb = work.tile([P, BPG, IN_DIM], BF16, tag="xnb")
                                nc.vector.tensor_copy(
                                    out=xnb[:], in_=xn_t[:, g * BPG : (g + 1) * BPG, :]
                                )
                                half = N_GRAPHS // 2
                                ohg = work.tile([P, BPG, N_GRAPHS], BF16, tag="ohg")
                                for h in (0, 1):
                                    nc.vector.tensor_tensor(
                                        out=ohg[:, :, h * half : (h + 1) * half],
                                        in0=gid_t[
                                            :, g * BPG : (g + 1) * BPG, h : h + 1
                                        ].to_broadcast([P, BPG, half]),
                                        in1=iota_t[:, None, :].to_broadcast(
                                            [P, BPG, half]
                                        ),
                                        op=mybir.AluOpType.is_equal,
                                    )
                                for bb in range(BPG):
                                    blk = g * BPG + bb
                                    nc.tensor.matmul(
                                        out=pool_ps[:],
                                        lhsT=xnb[:, bb, :],
                                        rhs=ohg[:, bb, :],
                                        start=(blk == 0),
                                        stop=(blk == NBLK - 1),
                                    )

                if mode == "B":
                    nc.sync.dma_start(out=x_out_d[:, :], in_=xn_t[:])
                    t_o = res.tile([P, NBLK, 10], BF16)
                    a_o = res.tile([P, NBLK, 10], BF16)
                    _table_build(nc, res, xn_t, tc_t, ac_t, ab_t, t_o, a_o)
                    nc.sync.dma_start(out=t_out_d[:, :], in_=t_o[:])
                    nc.sync.dma_start(out=a_out_d[:, :], in_=a_o[:])
                else:
                    # pooled partial sums -> scale -> AllReduce -> MLP
                    icnt_t = res.tile([P, N_GRAPHS], F32)
                    nc.sync.dma_start(out=icnt_t[:], in_=icnt_d[:, :])
                    psum_sb = res.tile([IN_DIM, N_GRAPHS], F32)
                    nc.vector.tensor_tensor(
                        out=psum_sb[:],
                        in0=pool_ps[:],
                        in1=icnt_t[0:IN_DIM, :],
                        op=mybir.AluOpType.mult,
                    )
                    with tc.tile_pool(name="dram", bufs=1, space="DRAM") as dpool:
                        cc_in = dpool.tile([IN_DIM, N_GRAPHS], F32)
                        cc_out = dpool.tile([IN_DIM, N_GRAPHS], F32)
                        nc.sync.dma_start(out=cc_in[:], in_=psum_sb[:])
                        nc.gpsimd.collective_compute(
                            "AllReduce",
                            mybir.AluOpType.add,
                            replica_groups=[list(range(NCORES))],
                            ins=[cc_in.opt()],
                            outs=[cc_out.opt()],
                        )
                        pooled_t = res.tile([IN_DIM, N_GRAPHS], F32)
                        nc.sync.dma_start(out=pooled_t[:], in_=cc_out[:])
                    w1t_t = res.tile([IN_DIM, HIDDEN], F32)
                    b1_t = res.tile([HIDDEN, 1], F32)
                    w2t_t = res.tile([HIDDEN, 1], F32)
                    b2_t = res.tile([1, 1], F32)
                    nc.sync.dma_start(out=w1t_t[:], in_=w1t_d[:, :])
                    nc.sync.dma_start(out=b1_t[:], in_=b1_d[:, :])
                    nc.sync.dma_start(out=w2t_t[:], in_=w2t_d[:, :])
                    nc.sync.dma_start(out=b2_t[:], in_=b2_d[:, :])
                    h_ps = psum.tile([HIDDEN, N_GRAPHS], F32)
                    nc.tensor.matmul(
                        out=h_ps[:], lhsT=w1t_t[:], rhs=pooled_t[:],
                        start=True, stop=True,
                    )
                    h_t = res.tile([HIDDEN, N_GRAPHS], F32)
                    nc.scalar.activation(
                        out=h_t[:], in_=h_ps[:],
                        func=mybir.ActivationFunctionType.Relu,
                        bias=b1_t[:],
                    )
                    o_ps = psum.tile([1, N_GRAPHS], F32)
                    nc.tensor.matmul(
                        out=o_ps[:], lhsT=w2t_t[:], rhs=h_t[:],
                        start=True, stop=True,
                    )
                    o_t = res.tile([1, N_GRAPHS], F32)
                    nc.vector.tensor_tensor(
                        out=o_t[:],
                        in0=o_ps[:],
                        in1=b2_t[:, 0:1].to_broadcast([1, N_GRAPHS]),
                        op=mybir.AluOpType.add,
                    )
                    nc.sync.dma_start(out=out_d[:, :], in_=o_t[:])
    return nc


def _preprocess(edge_index, edge_attr, batch):
    src = np.asarray(edge_index[0], dtype=np.int64)
    dst = np.asarray(edge_index[1], dtype=np.int64)
    attr = np.asarray(edge_attr, dtype=np.float32).reshape(-1)
    batch = np.asarray(batch, dtype=np.int64)

    deg = np.bincount(dst, minlength=N_NODES)
    eorder = np.argsort(dst, kind="stable")
    ssrc = src[eorder].astype(np.int32)
    sattr = attr[eorder]
    starts = np.zeros(N_NODES + 1, np.int64)
    np.cumsum(deg, out=starts[1:])

    cores = []
    for c in range(NCORES):
        lo = c * NPC
        degs = deg[lo : lo + NPC]
        order = np.argsort(-degs, kind="stable")  # local ids by degree desc
        node_arr = np.full(NPAD, -1, np.int64)
        node_arr[:NPC] = lo + order
        node_arr = node_arr.reshape(NBLK, P)  # [block, p]
        valid_n = node_arr >= 0
        nd = np.where(valid_n, deg[np.clip(node_arr, 0, None)], 0)

        dgs, src_chunks, attr_chunks = [], [], []
        for g in range(NGRP):
            nb = node_arr[g * BPG : (g + 1) * BPG]          # [BPG, P]
            db = nd[g * BPG : (g + 1) * BPG]                # [BPG, P]
            dg = max(1, int(db.max()))
            dgs.append(dg)
            j = np.arange(dg)
            eid = starts[np.clip(nb, 0, None)][:, :, None] + j  # [BPG,P,dg]
            ok = j[None, None, :] < db[:, :, None]
            sg = np.where(ok, ssrc[np.clip(eid, 0, N_EDGES - 1)], N_NODES)
            ag = np.where(ok, sattr[np.clip(eid, 0, N_EDGES - 1)], 0.0)
            src_chunks.append(sg.transpose(1, 0, 2).ravel())   # [P,BPG,dg]
            attr_chunks.append(ag.transpose(1, 0, 2).ravel().astype(NPBF16))

        slotsrc = np.concatenate(src_chunks)
        slotattr = np.concatenate(attr_chunks)
        invdeg = (1.0 / np.maximum(nd, 1)).astype(np.float32).reshape(NBLK, P).T
        invdeg = np.ascontiguousarray(invdeg)                  # [P, NBLK]
        gid = np.where(valid_n, batch[np.clip(node_arr, 0, None)], 0)
        gid = np.ascontiguousarray(gid.reshape(NBLK, P).T.astype(np.float32))
        cores.append(
            dict(
                node_arr=node_arr, dgs=dgs, slotsrc=slotsrc,
                slotattr=slotattr, invdeg=invdeg, gid=gid,
            )
        )
    cnt = np.bincount(batch, minlength=N_GRAPHS).astype(np.float32)
    invcnt = (1.0 / np.maximum(cnt, 1.0)).astype(np.float32)
    return cores, invcnt


def _node_layout(arr_full, node_arr, width):
    """[N_NODES(+), width] full-array -> per-core [P, NBLK*width] layout."""
    out = np.zeros((NBLK, P, width), np.float32)
    v = node_arr >= 0
    out[v] = arr_full[node_arr[v]]
    return np.ascontiguousarray(out.transpose(1, 0, 2).reshape(P, NBLK * width))


def _unlayout(per_core, node_arr, width):
    """[P, NBLK*width] device layout -> rows in original node ids (full)."""
    a = per_core.reshape(P, NBLK, width).transpose(1, 0, 2)  # [NBLK, P, w]
    return a, node_arr


def kernel(x, edge_index, edge_attr, batch, **params):
    x = np.asarray(x, dtype=np.float32)
    cores, invcnt = _preprocess(edge_index, edge_attr, batch)
    wpacks = _build_wpacks(params)

    dgs = cores[0]["dgs"]
    # all cores must share one compiled program -> unify group sizes
    max_dgs = [max(c["dgs"][g] for c in cores) for g in range(NGRP)]
    for c in cores:
        if c["dgs"] != max_dgs:
            # repack with padded group sizes
            c_new_src, c_new_attr, off = [], [], 0
            for g, (dg_c, dg_m) in enumerate(zip(c["dgs"], max_dgs)):
                n = P * BPG * dg_c
                sg = c["slotsrc"][off : off + n].reshape(P, BPG, dg_c)
                ag = c["slotattr"][off : off + n].reshape(P, BPG, dg_c)
                off += n
                if dg_c < dg_m:
                    sg2 = np.full((P, BPG, dg_m), N_NODES, np.int32)
                    ag2 = np.zeros((P, BPG, dg_m), NPBF16)
                    sg2[:, :, :dg_c] = sg
                    ag2[:, :, :dg_c] = ag
                    sg, ag = sg2, ag2
                c_new_src.append(sg.ravel())
                c_new_attr.append(ag.ravel())
            c["slotsrc"] = np.concatenate(c_new_src)
            c["slotattr"] = np.concatenate(c_new_attr)
            c["dgs"] = list(max_dgs)
    dgs = max_dgs
    S = sum(P * BPG * d for d in dgs)
    for c in cores:
        c["at10"] = np.repeat(c["slotattr"], 10)

    key = tuple(dgs)
    if key not in _cache:
        _cache[key] = (
            _build_program("A", dgs),
            _build_program("B", dgs),
            _build_program("B3", dgs),
        )
    nc_a, nc_b, nc_b3 = _cache[key]

    import os

    core_ids = list(range(NCORES))
    if bool(int(os.environ.get("CGCNN_TRACE", "0"))):
        from concourse.timeline_sim import TimelineSim

        est = []
        for name, prog in (("A", nc_a), ("B", nc_b), ("B3", nc_b3)):
            try:
                ns = TimelineSim(prog).simulate()
            except Exception as e:
                ns = None
                print(f"TimelineSim {name} failed: {e}")
            est.append((name, ns))
        kernel.timeline_est = est
        print("cost-model timeline estimate per launch (ns):", est)
    rep = np.ones((P, 1), np.float32)

    def wp_maps(l):
        wp = wpacks[l]
        return {
            "tcoef": np.ascontiguousarray((rep * wp["tc"].reshape(1, -1))),
            "acoef": np.ascontiguousarray((rep * wp["ac"].reshape(1, -1))),
            "abias": np.ascontiguousarray((rep * wp["abias"].reshape(1, -1))),
        }

    # ---- launch A: initial tables from x ----
    in_maps = []
    for c in cores:
        m = {"x": _node_layout(x, c["node_arr"], IN_DIM)}
        m.update(wp_maps(0))
        in_maps.append(m)
    res_a = run_bass_kernel_spmd(nc_a, in_maps, core_ids=core_ids)

    exec_times = [res_a.exec_time_ns]
    trace_paths = []
    if res_a.instructions_and_trace is not None:
        trace_paths.append(res_a.instructions_and_trace[1])

    def assemble_tables(res):
        t_full = np.full((N_NODES + 1, 10), PAD_VAL, NPBF16)
        a_slices = []
        for ci, c in enumerate(cores):
            t_pc, node_arr = _unlayout(res.results[ci]["t_next"], c["node_arr"], 10)
            v = node_arr >= 0
            t_full[node_arr[v]] = t_pc[v]
            a_slices.append(np.ascontiguousarray(res.results[ci]["a_next"]))
        return t_full, a_slices

    t_full, a_slices = assemble_tables(res_a)

    x_cur = [in_maps[c]["x"] for c in range(NCORES)]

    for l in (1, 2, 3):
        last = l == 3
        in_maps = []
        for ci, c in enumerate(cores):
            gat = np.ascontiguousarray(t_full[c["slotsrc"]]).ravel()
            wp = wpacks[l - 1]
            m = {
                "x": x_cur[ci],
                "gat": gat,
                "at10": c["at10"],
                "adst": a_slices[ci],
                "invdeg": c["invdeg"],
                "wecur": np.ascontiguousarray(
                    (rep * wp["we"].reshape(1, -1))
                ).astype(NPBF16),
            }
            if not last:
                m.update(wp_maps(l))
            else:
                g2 = np.stack([c["gid"], c["gid"] - 256.0], axis=-1)
                m["gid2"] = np.ascontiguousarray(
                    g2.reshape(P, NBLK * 2)
                ).astype(NPBF16)
                m["iota"] = np.ascontiguousarray(
                    rep * np.arange(N_GRAPHS // 2, dtype=np.float32).reshape(1, -1)
                ).astype(NPBF16)
                m["invcnt"] = np.ascontiguousarray(
                    rep * invcnt.reshape(1, -1)
                )
                m["w1t"] = np.ascontiguousarray(params["W1"].T.astype(np.float32))
                m["b1"] = np.ascontiguousarray(
                    params["b1"].astype(np.float32).reshape(HIDDEN, 1)
                )
                m["w2t"] = np.ascontiguousarray(params["W2"].T.astype(np.float32))
                m["b2"] = np.ascontiguousarray(
                    params["b2"].astype(np.float32).reshape(1, 1)
                )
            in_maps.append(m)
        res = run_bass_kernel_spmd(
            nc_b3 if last else nc_b, in_maps, core_ids=core_ids
        )
        exec_times.append(res.exec_time_ns)
        if res.instructions_and_trace is not None:
            trace_paths.append(res.instructions_and_trace[1])
        if not last:
            t_full, a_slices = assemble_tables(res)
            x_cur = [res.results[ci]["x_next"] for ci in range(NCORES)]

    kernel.exec_times = exec_times
    kernel.trace_paths = trace_paths
    out = res.results[0]["out"].reshape(N_GRAPHS, 1).astype(np.float32)
    return out

